# revision 2
# baseline (speedup 1.0000x reference)
"""Bi-directional MinGRU kernel for Trainium2 (8 NeuronCores, SPMD).

Problem: x [4, 4096, 1024]; per direction d in {fwd, bwd}:
    k  = x @ Wz_d + bz_d
    A  = sigmoid(-k)           (= 1 - z, the carry coefficient)
    z  = sigmoid(k)
    gp = x @ Wh_d + bh_d
    g  = max(gp + 0.5, sigmoid(gp))      (== where(gp>=0, gp+0.5, sigmoid(gp)))
    h_t = A_t * h_{t-1} + z_t * g_t      (linear first-order scan over S)
    out = concat(h_fwd, h_bwd) @ W_out + b_out

Sharding: 8 cores = (4 batches) x (2 directions). Each core computes the
full hidden state for one (batch, direction) and its half of the final
2H->H projection; the two partial products per batch are summed on host.

Per-core layout: everything is kept transposed ([channel, seq]) so the
sequential scan runs along the free dimension with channels on partitions,
using the native VectorE tensor_tensor_scan instruction.
"""

import os
import numpy as np
from contextlib import ExitStack

import concourse.bass as bass
import concourse.tile as tile
from concourse import bacc, mybir
from concourse.bass_utils import run_bass_kernel_spmd

P = 128          # partitions
S = 4096         # sequence length
D = 1024         # input dim
H = 1024         # hidden dim
SC = 512         # seq chunk (one PSUM bank of fp32)
NSC = S // SC    # 8 seq chunks
ND = D // P      # 8 contraction tiles for GEMM1
NH = H // P      # 8 hidden tiles
NCORES = 8

F32 = mybir.dt.float32

# matmul input mode: "f32r" (fp32 data, 1 cyc/row PE path), "bf16", "f32"
MM_MODE = os.environ.get("BIMINGRU_MM_MODE", "f32r")

if MM_MODE == "bf16":
    IN_DT = mybir.dt.bfloat16
    H_DT = mybir.dt.bfloat16     # scan output dtype (GEMM3 rhs)
elif MM_MODE == "f32r":
    # float32r must be declared end-to-end (walrus birverifier requires the
    # producer chain to be f32r-typed); the raw bytes are plain fp32.
    IN_DT = mybir.dt.float32r
    H_DT = mybir.dt.float32r
else:
    IN_DT = F32
    H_DT = F32


def _np_in_dt():
    if MM_MODE == "bf16":
        import ml_dtypes
        return np.dtype(ml_dtypes.bfloat16)
    return np.dtype(np.float32)


def _mm(ap):
    return ap


def _build_module():
    nc = bacc.Bacc("TRN2", target_bir_lowering=False, debug=False)

    xT = nc.dram_tensor("xT", [D, S], IN_DT, kind="ExternalInput").ap()
    Wz = nc.dram_tensor("Wz", [D, H], IN_DT, kind="ExternalInput").ap()
    Wh = nc.dram_tensor("Wh", [D, H], IN_DT, kind="ExternalInput").ap()
    Wo = nc.dram_tensor("Wo", [H, H], IN_DT, kind="ExternalInput").ap()
    bz = nc.dram_tensor("bz", [H], F32, kind="ExternalInput").ap()
    bh = nc.dram_tensor("bh", [H], F32, kind="ExternalInput").ap()
    outT = nc.dram_tensor("outT", [H, S], F32, kind="ExternalOutput").ap()

    AF = mybir.ActivationFunctionType
    OP = mybir.AluOpType

    with tile.TileContext(nc) as tc, ExitStack() as ctx:
        wpool = ctx.enter_context(tc.tile_pool(name="w", bufs=1))
        xpool = ctx.enter_context(tc.tile_pool(name="x", bufs=2))
        pspool = ctx.enter_context(tc.tile_pool(name="ps", bufs=2, space="PSUM"))
        ewpool = ctx.enter_context(tc.tile_pool(name="ew", bufs=2))
        hpool = ctx.enter_context(tc.tile_pool(name="h", bufs=2))
        opool = ctx.enter_context(tc.tile_pool(name="o", bufs=3))

        # --- weights: resident in SBUF for the whole kernel ---
        Wz_t, Wh_t, Wo_t = [], [], []
        for d in range(ND):
            wzt = wpool.tile([P, H], IN_DT, tag=f"wz{d}", name=f"wz{d}")
            nc.sync.dma_start(wzt[:], Wz[d * P:(d + 1) * P, :])
            Wz_t.append(wzt)
            wht = wpool.tile([P, H], IN_DT, tag=f"wh{d}", name=f"wh{d}")
            nc.sync.dma_start(wht[:], Wh[d * P:(d + 1) * P, :])
            Wh_t.append(wht)
        for i in range(NH):
            wot = wpool.tile([P, H], IN_DT, tag=f"wo{i}", name=f"wo{i}")
            nc.sync.dma_start(wot[:], Wo[i * P:(i + 1) * P, :])
            Wo_t.append(wot)

        # --- biases as per-partition columns: sb[p, i] = b[i*128 + p] ---
        bz_sb = wpool.tile([P, NH], F32, tag="bz", name="bz_sb")
        nc.sync.dma_start(bz_sb[:], bz.rearrange("(j p) -> p j", p=P))
        bh_sb = wpool.tile([P, NH], F32, tag="bh", name="bh_sb")
        nc.sync.dma_start(bh_sb[:], bh.rearrange("(j p) -> p j", p=P))
        nbz_sb = wpool.tile([P, NH], F32, tag="nbz", name="nbz_sb")
        nc.vector.tensor_scalar_mul(nbz_sb[:], bz_sb[:], -1.0)
        bh5_sb = wpool.tile([P, NH], F32, tag="bh5", name="bh5_sb")
        nc.vector.tensor_scalar_add(bh5_sb[:], bh_sb[:], 0.5)

        h_tiles = [[None] * NH for _ in range(NSC)]

        def emit_gemm1_chunk(j):
            # x chunk [d][128, SC]
            xc = []
            for d in range(ND):
                xt = xpool.tile([P, SC], IN_DT, tag=f"x{d}", name=f"x{d}_{j}")
                nc.sync.dma_start(xt[:], xT[d * P:(d + 1) * P, j * SC:(j + 1) * SC])
                xc.append(xt)
            for i in range(NH):
                psK = pspool.tile([P, SC], F32, tag="psK", name=f"psK_{j}_{i}")
                psG = pspool.tile([P, SC], F32, tag="psG", name=f"psG_{j}_{i}")
                for d in range(ND):
                    nc.tensor.matmul(
                        psK[:], _mm(Wz_t[d][:, i * P:(i + 1) * P]), _mm(xc[d][:]),
                        start=(d == 0), stop=(d == ND - 1))
                for d in range(ND):
                    nc.tensor.matmul(
                        psG[:], _mm(Wh_t[d][:, i * P:(i + 1) * P]), _mm(xc[d][:]),
                        start=(d == 0), stop=(d == ND - 1))

                A = ewpool.tile([P, SC], F32, tag="A", name=f"A_{j}_{i}")
                nc.scalar.activation(A[:], psK[:], AF.Sigmoid,
                                     bias=nbz_sb[:, i:i + 1], scale=-1.0)
                z = ewpool.tile([P, SC], F32, tag="z", name=f"z_{j}_{i}")
                nc.scalar.activation(z[:], psK[:], AF.Sigmoid,
                                     bias=bz_sb[:, i:i + 1], scale=1.0)
                sg = ewpool.tile([P, SC], F32, tag="sg", name=f"sg_{j}_{i}")
                nc.scalar.activation(sg[:], psG[:], AF.Sigmoid,
                                     bias=bh_sb[:, i:i + 1], scale=1.0)
                g = ewpool.tile([P, SC], F32, tag="g", name=f"g_{j}_{i}")
                nc.vector.scalar_tensor_tensor(g[:], psG[:], bh5_sb[:, i:i + 1],
                                               sg[:], op0=OP.add, op1=OP.max)
                Bv = ewpool.tile([P, SC], F32, tag="B", name=f"B_{j}_{i}")
                nc.vector.tensor_tensor(Bv[:], z[:], g[:], op=OP.mult)

                ht = hpool.tile([P, SC], H_DT, tag=f"h{i}", name=f"h_{j}_{i}")
                init = 0.0 if j == 0 else h_tiles[j - 1][i][:, SC - 1:SC]
                nc.vector.tensor_tensor_scan(ht[:], A[:], Bv[:], initial=init,
                                             op0=OP.mult, op1=OP.add)
                h_tiles[j][i] = ht

        def emit_gemm3_chunk(j):
            for o in range(NH):
                psO = pspool.tile([P, SC], F32, tag="psO", name=f"psO_{j}_{o}")
                for i in range(NH):
                    nc.tensor.matmul(
                        psO[:], _mm(Wo_t[i][:, o * P:(o + 1) * P]),
                        _mm(h_tiles[j][i][:]),
                        start=(i == 0), stop=(i == NH - 1))
                oc = opool.tile([P, SC], F32, tag="oc", name=f"oc_{j}_{o}")
                nc.scalar.copy(oc[:], psO[:])
                nc.sync.dma_start(outT[o * P:(o + 1) * P, j * SC:(j + 1) * SC], oc[:])

        # software pipeline: GEMM3 for chunk j-1 is emitted after GEMM1 for
        # chunk j so the PE never waits on the scans of the current chunk
        for j in range(NSC):
            emit_gemm1_chunk(j)
            if j >= 1:
                emit_gemm3_chunk(j - 1)
        emit_gemm3_chunk(NSC - 1)

    nc.compile()
    return nc


_CACHE = {}


def _get_module():
    if "nc" not in _CACHE:
        _CACHE["nc"] = _build_module()
    return _CACHE["nc"]


def _make_in_maps(x, Wz_f, bz_f, Wh_f, bh_f, Wz_b, bz_b, Wh_b, bh_b, W_out, b_out):
    np_in = _np_in_dt()
    f32 = np.float32

    def prep_w(w):
        return np.ascontiguousarray(np.asarray(w), dtype=np_in)

    x = np.asarray(x, dtype=f32)
    Wz_fc, Wh_fc = prep_w(Wz_f), prep_w(Wh_f)
    Wz_bc, Wh_bc = prep_w(Wz_b), prep_w(Wh_b)
    W_out = np.asarray(W_out)
    Wo_fc = prep_w(W_out[:H])      # fwd half rows of W_out
    Wo_bc = prep_w(W_out[H:])      # bwd half rows
    bz_fc = np.ascontiguousarray(np.asarray(bz_f), dtype=f32)
    bh_fc = np.ascontiguousarray(np.asarray(bh_f), dtype=f32)
    bz_bc = np.ascontiguousarray(np.asarray(bz_b), dtype=f32)
    bh_bc = np.ascontiguousarray(np.asarray(bh_b), dtype=f32)

    in_maps = []
    for b in range(4):
        xT_f = np.ascontiguousarray(x[b].T, dtype=np_in)          # [D, S]
        xT_b = np.ascontiguousarray(x[b, ::-1].T, dtype=np_in)    # reversed seq
        in_maps.append({"xT": xT_f, "Wz": Wz_fc, "Wh": Wh_fc, "Wo": Wo_fc,
                        "bz": bz_fc, "bh": bh_fc})
        in_maps.append({"xT": xT_b, "Wz": Wz_bc, "Wh": Wh_bc, "Wo": Wo_bc,
                        "bz": bz_bc, "bh": bh_bc})
    return in_maps


def _assemble(results, b_out):
    out = np.empty((4, S, H), np.float32)
    for b in range(4):
        out[b] = results[2 * b]["outT"].T
        out[b] += results[2 * b + 1]["outT"].T
    out += np.asarray(b_out, dtype=np.float32)
    return out


def kernel(x, Wz_f, bz_f, Wh_f, bh_f, Wz_b, bz_b, Wh_b, bh_b, W_out, b_out):
    nc = _get_module()
    in_maps = _make_in_maps(x, Wz_f, bz_f, Wh_f, bh_f,
                            Wz_b, bz_b, Wh_b, bh_b, W_out, b_out)
    res = run_bass_kernel_spmd(nc, in_maps, core_ids=list(range(NCORES)))
    return _assemble(res.results, b_out)
